# revision 1
# baseline (speedup 1.0000x reference)
"""CrossModalAttention TRN2 kernel.

Strategy (data-parallel over batch, one batch element per NeuronCore):
  dir a: q from rgb, k/v from pl;  dir b: q from pl, k/v from rgb.
  Per direction:
    Q  = scale*(Wq @ f_q + bq)        [128 e, N]   (scale folded into Q)
    K  = Wk @ f_k + bk                [128 e, N]
    VT = (Wv @ f_k)^T = f_k^T @ Wv^T  [N k, 128 e] (no bias; folded into BN shift)
    per q-tile (512 wide):
      S^T_j = K_j^T @ Q_tile          [128 k, 512 q]  per k-chunk j (PSUM)
      E_j   = exp(S^T_j)              (ScalarE eviction, exp_and_others table)
      OT   += VT_j^T @ E_j            [128 e, 512 q]  (PSUM accumulate over j)
      dn   += ones^T @ E_j            [1, 512 q]      (softmax denominator)
      OT_norm = OT * bcast(1/dn)      (bcast via Kc=1 rank-1 matmul)
  y = Wp_a @ OT_a + Wp_b @ OT_b ; out = relu(inv*y + shift)  (BN folded, incl.
  v-bias contribution: shift' = beta - mean*inv + inv*(Wp_a@bv_a + Wp_b@bv_b))
"""

import sys

sys.path.insert(0, "/opt/trn_rl_repo")

import numpy as np

B = 8
C = 256
E = 128
OUT = 256
H = W = 64
N = H * W
QW = 512
SCALE = float(E) ** -0.5

_CACHE = {}


def _patch_tail_drain(tile_mod, mybir):
    # This walrus build encodes Drain as CTRL_NO_STRUCT with a single
    # sync-wait slot; split the TileContext tail drain's waits across
    # one drain instruction per semaphore.
    if getattr(tile_mod.TileContext, "_drain_patched", False):
        return
    from concourse.vector_clock import ScopedClock

    def _drain_and_barrier(self, tick_clock, wait_clock):
        nc = self.nc
        drain_inst = nc.sync.drain()
        wait_clock.add_sem_waits(
            drain_inst.ins, ScopedClock({None: tick_clock.global_clock})
        )
        si = drain_inst.ins.sync_info
        if si is not None and si.on_wait and len(si.on_wait) > 1:
            waits = list(si.on_wait)
            drain_inst.ins.sync_info = mybir.SyncInfo(
                on_wait=[waits[0]], on_update=list(si.on_update or [])
            )
            for w in waits[1:]:
                d2 = nc.sync.drain()
                d2.ins.sync_info = mybir.SyncInfo(on_wait=[w], on_update=[])
        nc.all_engine_barrier()
        popped = nc._tile_sem_poison_stack.pop()
        assert popped is self._sem_poison
        nc.clear_and_free_semaphores(list(self.sems.allocated().values()))
        nc.all_engine_barrier()

    tile_mod.TileContext._drain_and_barrier = _drain_and_barrier
    tile_mod.TileContext._drain_patched = True


def build_nc(n=N, gj=2, debug=False):
    """Build the single-core Bass program. n = spatial size (4096 full)."""
    import concourse.bacc as bacc
    import concourse.tile as tile
    from concourse import mybir

    f32 = mybir.dt.float32
    f32r = mybir.dt.float32r
    AFT = mybir.ActivationFunctionType

    nqt = n // QW
    nkc = n // 128
    ngrp = nkc // gj

    nc = bacc.Bacc(trn_type="TRN2", target_bir_lowering=False, debug=False)

    def din(name, shape):
        return nc.dram_tensor(name, shape, f32, kind="ExternalInput").ap()

    f_a_d = din("f_a", [C, n])        # rgb features (q-side of dir a)
    f_b_d = din("f_b", [C, n])        # pl features
    wq_a_d = din("wq_a", [C, E])      # W_q_rgb^T
    wk_a_d = din("wk_a", [C, E])      # W_k_pl^T
    wv_a_d = din("wv_a", [C, E])      # W_v_pl^T
    wq_b_d = din("wq_b", [C, E])      # W_q_pl^T
    wk_b_d = din("wk_b", [C, E])      # W_k_rgb^T
    wv_b_d = din("wv_b", [C, E])      # W_v_rgb^T
    wp_d = din("wp", [2 * E, OUT])    # w_proj^T
    bq_a_d = din("bq_a", [E, 1])      # scale * b_q_rgb
    bk_a_d = din("bk_a", [E, 1])      # b_k_pl
    bq_b_d = din("bq_b", [E, 1])      # scale * b_q_pl
    bk_b_d = din("bk_b", [E, 1])      # b_k_rgb
    inv_d = din("bn_inv", [OUT, 1])
    shf_d = din("bn_shf", [OUT, 1])
    ones_c_d = din("ones_c", [E, 1])
    ones_r_d = din("ones_r", [1, E])
    ident_d = din("ident", [E, E])
    y_d = nc.dram_tensor("y", [OUT, n], f32, kind="ExternalOutput").ap()
    dbg = {}
    if debug:
        for nm in ("dq0", "dk0", "dv0", "dot0", "ddn0"):
            shp = [1, QW] if nm == "ddn0" else ([128, QW] if nm == "dot0" else [128, n])
            dbg[nm] = nc.dram_tensor(nm, shp, f32, kind="ExternalOutput").ap()

    with tile.TileContext(nc) as tc:
        with tc.tile_pool(name="const", bufs=1) as const, \
             tc.tile_pool(name="qkv", bufs=1) as qkv, \
             tc.tile_pool(name="pst", bufs=3, space="PSUM") as pst, \
             tc.tile_pool(name="pot", bufs=1, space="PSUM") as pot, \
             tc.tile_pool(name="pshared", bufs=1, space="PSUM") as pshared:
            # ---- constants (DMA order: critical-path first) ----
            def wload(d, nm):
                t = const.tile([128, 2, E], f32r, name=nm, tag=nm)
                nc.sync.dma_start(t[:], d.rearrange("(c p) e -> p c e", p=128).bitcast(f32r))
                return t

            def vload(d, shape, nm, dt_=None):
                t = const.tile(shape, dt_ or f32, name=nm, tag=nm)
                nc.sync.dma_start(t[:], d.bitcast(dt_) if dt_ else d)
                return t

            wq = {0: wload(wq_a_d, "wqa"), 1: wload(wq_b_d, "wqb")}
            wk = {0: wload(wk_a_d, "wka"), 1: wload(wk_b_d, "wkb")}
            bq = {0: vload(bq_a_d, [E, 1], "bqa"), 1: vload(bq_b_d, [E, 1], "bqb")}
            bk = {0: vload(bk_a_d, [E, 1], "bka"), 1: vload(bk_b_d, [E, 1], "bkb")}

            # ---- per-direction activations ----
            q_sb = {d: qkv.tile([128, n], f32r, tag=f"q{d}", name=f"q_sb{d}") for d in (0, 1)}
            k_sb = {d: qkv.tile([128, n], f32r, tag=f"k{d}", name=f"k_sb{d}") for d in (0, 1)}
            vt_sb = {d: qkv.tile([128, n], f32r, tag=f"v{d}", name=f"vt_sb{d}") for d in (0, 1)}

            # ---- feature load + projections (feature pool freed after) ----
            with tc.tile_pool(name="feat", bufs=1) as feat:
                fsb = {
                    name: feat.tile([128, 2, n], f32r, tag=f"f{name}",
                                    name=f"f_{name}")
                    for name in ("a", "b")
                }
                npc = max(1, n // 1024)   # ~1MB DMA pieces
                wv = {}
                for pc in range(npc):
                    lo, hi = pc * (n // npc), (pc + 1) * (n // npc)
                    for cc in range(2):
                        for name, d in (("a", f_a_d), ("b", f_b_d)):
                            eng = nc.sync if (cc == 0) else nc.gpsimd
                            eng.dma_start(
                                fsb[name][:, cc, lo:hi],
                                d[cc * 128:(cc + 1) * 128, lo:hi].bitcast(f32r),
                            )
                    if pc == min(1, npc - 1):
                        wv = {0: wload(wv_a_d, "wva"), 1: wload(wv_b_d, "wvb")}
                # late consts (used mid/late in the kernel)
                wp = const.tile([128, 2, OUT], f32r, name="wp", tag="wp")
                nc.sync.dma_start(wp[:], wp_d.rearrange("(c p) e -> p c e", p=128).bitcast(f32r))
                binv = const.tile([128, 2, 1], f32, name="binv", tag="binv")
                nc.sync.dma_start(binv[:], inv_d.rearrange("(c p) e -> p c e", p=128))
                bshf = const.tile([128, 2, 1], f32, name="bshf", tag="bshf")
                nc.sync.dma_start(bshf[:], shf_d.rearrange("(c p) e -> p c e", p=128))
                ones_c = vload(ones_c_d, [E, 1], "onc", f32r)
                ones_r = vload(ones_r_d, [1, E], "onr", f32r)
                ident = vload(ident_d, [E, E], "idt", f32r)

                vtmps = {}
                for d in (0, 1):
                    fq = fsb["a"] if d == 0 else fsb["b"]
                    fk = fsb["b"] if d == 0 else fsb["a"]
                    # Q and K projections: [128 e, n]
                    for which, wt, bias, scl, dst, src_f in (
                        ("q", wq[d], bq[d], SCALE, q_sb[d], fq),
                        ("k", wk[d], bk[d], 1.0, k_sb[d], fk),
                    ):
                        for nt in range(nqt):
                            ps = pst.tile([128, QW], f32, tag="st", name="psqk")
                            for cc in range(2):
                                nc.tensor.matmul(
                                    ps[:],
                                    wt[:, cc, :],
                                    src_f[:, cc, nt * QW:(nt + 1) * QW],
                                    start=(cc == 0),
                                    stop=(cc == 1),
                                )
                            nc.scalar.activation(
                                dst[:, nt * QW:(nt + 1) * QW], ps[:],
                                AFT.Identity, bias=bias[:], scale=scl,
                            )
                    # V = Wv @ fk  [128 e, n]
                    vtmp = feat.tile([128, n], f32r, tag=f"vtmp{d}",
                                     name=f"vtmp{d}")
                    vtmps[d] = vtmp
                    for nt in range(nqt):
                        ps = pst.tile([128, QW], f32, tag="st", name="psv")
                        for cc in range(2):
                            nc.tensor.matmul(
                                ps[:],
                                wv[d][:, cc, :],
                                fk[:, cc, nt * QW:(nt + 1) * QW],
                                start=(cc == 0),
                                stop=(cc == 1),
                            )
                        with nc.allow_low_precision(reason="f32r V"):
                            nc.vector.tensor_copy(
                                vtmp[:, nt * QW:(nt + 1) * QW], ps[:]
                            )
                # PE transposes after both dirs' projections (DVE copies hide)
                for d in (0, 1):
                    for g in range(nkc // 4):
                        ps = pst.tile([128, 512], f32r, tag="st", name="psvt")
                        for jj in range(4):
                            kc = 4 * g + jj
                            nc.tensor.transpose(
                                ps[:, jj * 128:(jj + 1) * 128],
                                vtmps[d][:, kc * 128:(kc + 1) * 128],
                                ident[:],
                            )
                        with nc.allow_low_precision(reason="f32r VT"):
                            nc.vector.tensor_copy(
                                vt_sb[d][:, g * 512:(g + 1) * 512], ps[:]
                            )
            if debug:
                nc.sync.dma_start(dbg["dq0"].bitcast(f32r), q_sb[0][:])
                nc.sync.dma_start(dbg["dk0"].bitcast(f32r), k_sb[0][:])
                nc.sync.dma_start(dbg["dv0"].bitcast(f32r), vt_sb[0][:])

            # ---- attention + output ----
            with tc.tile_pool(name="sex", bufs=4) as sex, \
                 tc.tile_pool(name="sot", bufs=3) as sot, \
                 tc.tile_pool(name="smisc", bufs=4) as smisc:

                def emit_S(d, qt, g):
                    """S^T matmuls for one k-chunk group -> st psum tile."""
                    qs = q_sb[d][:, qt * QW:(qt + 1) * QW]
                    st = pst.tile([128, gj, QW], f32, tag="st", name="st")
                    for jj in range(gj):
                        j = gj * g + jj
                        nc.tensor.matmul(
                            st[:, jj, :],
                            k_sb[d][:, j * 128:(j + 1) * 128],
                            qs,
                            start=True, stop=True,
                        )
                    return st

                segs = [(qt, d) for qt in range(nqt) for d in (0, 1)]

                def emit_body(d, qt, st0):
                    """exp + O/dn accumulation for one (qt, d); st0 is the
                    pre-emitted group-0 S tile. Returns (ot, dn) psums."""
                    ot = pot.tile([128, QW], f32, tag="ot", name="ot")
                    dn = pshared.tile([1, QW], f32, tag="sh", name="dn")
                    st_cur = st0
                    for g in range(ngrp):
                        st_next = emit_S(d, qt, g + 1) if g + 1 < ngrp else None
                        ex = sex.tile([128, gj, QW], f32r, tag="ex", name="ex")
                        nc.scalar.activation(ex[:], st_cur[:], AFT.Exp)
                        # pair-sum on DVE, quad-sum on GPSIMD: PE only sees
                        # one denominator matmul per 4 k-chunks
                        exs = sex.tile([128, QW], f32r, tag="exs", name="exs")
                        with nc.allow_low_precision(reason="f32r denom"):
                            nc.vector.tensor_add(exs[:], ex[:, 0, :], ex[:, 1, :])
                        for jj in range(gj):
                            j = gj * g + jj
                            nc.tensor.matmul(
                                ot[:],
                                vt_sb[d][:, j * 128:(j + 1) * 128],
                                ex[:, jj, :],
                                start=(j == 0), stop=(j == nkc - 1),
                            )
                        nc.tensor.matmul(
                            dn[:],
                            ones_c[:],
                            exs[:],
                            start=(g == 0), stop=(g == ngrp - 1),
                        )
                        st_cur = st_next
                    if debug and qt == 0 and d == 0:
                        dot_sb = smisc.tile([128, QW], f32, tag="dbgot")
                        nc.vector.tensor_copy(dot_sb[:], ot[:])
                        nc.sync.dma_start(dbg["dot0"][:], dot_sb[:])
                        ddn_sb = smisc.tile([1, QW], f32, tag="dbgdn")
                        nc.vector.tensor_copy(ddn_sb[:], dn[:])
                        nc.sync.dma_start(dbg["ddn0"][:], ddn_sb[:])
                    return ot, dn

                def emit_tail(ot, dn):
                    """softmax normalize -> osb (f32r SBUF)."""
                    rc = smisc.tile([1, QW], f32r, tag="rc", name="rc")
                    with nc.allow_low_precision(reason="f32r feeds PE bcast"):
                        nc.vector.reciprocal(rc[:], dn[:])
                    bc_ps = pshared.tile([128, QW], f32, tag="sh", name="bc")
                    nc.tensor.matmul(
                        bc_ps[:], ones_r[:], rc[:], start=True, stop=True,
                    )
                    bc_sb = smisc.tile([128, QW], f32r, tag="bcs", name="bcs")
                    nc.vector.tensor_copy(bc_sb[:], bc_ps[:])
                    osb = sot.tile([128, QW], f32r, tag="osb", name="osb")
                    nc.vector.tensor_mul(osb[:], ot[:], bc_sb[:])
                    return osb

                def emit_final(qt, ot_sbs):
                    for dch in range(2):
                        yp = pshared.tile([128, QW], f32, tag="sh", name="yp")
                        for d in (0, 1):
                            nc.tensor.matmul(
                                yp[:],
                                wp[:, d, dch * 128:(dch + 1) * 128],
                                ot_sbs[d][:],
                                start=(d == 0), stop=(d == 1),
                            )
                        ysb = smisc.tile([128, QW], f32, tag="ysb", name="ysb")
                        nc.scalar.activation(
                            ysb[:], yp[:], AFT.Relu,
                            bias=bshf[:, dch, :], scale=binv[:, dch, :],
                        )
                        nc.sync.dma_start(
                            y_d[dch * 128:(dch + 1) * 128,
                                qt * QW:(qt + 1) * QW],
                            ysb[:],
                        )

                # software-pipelined over segments: the next segment's first
                # S-group is emitted before the current segment's tail so the
                # PE never idles on the DVE/ACT tail chain.
                st_next0 = emit_S(segs[0][1], segs[0][0], 0)
                pending = {}          # qt -> {d: osb}
                for i, (qt, d) in enumerate(segs):
                    ot, dn = emit_body(d, qt, st_next0)
                    if i + 1 < len(segs):
                        nqt_, nd_ = segs[i + 1]
                        st_next0 = emit_S(nd_, nqt_, 0)
                    osb = emit_tail(ot, dn)
                    pending.setdefault(qt, {})[d] = osb
                    if d == 1:
                        emit_final(qt, pending.pop(qt))
    nc.compile()
    return nc


def _host_prep(inputs, n=N):
    f_rgb = np.ascontiguousarray(
        inputs["f_rgb"].reshape(B, C, n).astype(np.float32))
    f_pl = np.ascontiguousarray(
        inputs["f_pl"].reshape(B, C, n).astype(np.float32))

    def T(w):
        return np.ascontiguousarray(np.asarray(w, np.float32).T)

    wp = np.asarray(inputs["w_proj"], np.float32)
    inv = np.asarray(inputs["bn_gamma"], np.float32) / np.sqrt(
        np.asarray(inputs["bn_var"], np.float32) + 1e-5)
    shift = (np.asarray(inputs["bn_beta"], np.float32)
             - np.asarray(inputs["bn_mean"], np.float32) * inv
             + inv * (wp[:, :E] @ np.asarray(inputs["b_v_pl"], np.float32)
                      + wp[:, E:] @ np.asarray(inputs["b_v_rgb"], np.float32)))

    shared = {
        "wq_a": T(inputs["w_q_rgb"]),
        "wk_a": T(inputs["w_k_pl"]),
        "wv_a": T(inputs["w_v_pl"]),
        "wq_b": T(inputs["w_q_pl"]),
        "wk_b": T(inputs["w_k_rgb"]),
        "wv_b": T(inputs["w_v_rgb"]),
        "wp": T(wp),
        "bq_a": (SCALE * np.asarray(inputs["b_q_rgb"], np.float32))
        .reshape(E, 1).copy(),
        "bk_a": np.asarray(inputs["b_k_pl"], np.float32).reshape(E, 1).copy(),
        "bq_b": (SCALE * np.asarray(inputs["b_q_pl"], np.float32))
        .reshape(E, 1).copy(),
        "bk_b": np.asarray(inputs["b_k_rgb"], np.float32).reshape(E, 1).copy(),
        "bn_inv": inv.reshape(OUT, 1).copy(),
        "bn_shf": shift.reshape(OUT, 1).copy(),
        "ones_c": np.ones((E, 1), np.float32),
        "ones_r": np.ones((1, E), np.float32),
        "ident": np.eye(E, dtype=np.float32),
    }
    in_maps = []
    for b in range(B):
        m = dict(shared)
        m["f_a"] = f_rgb[b]
        m["f_b"] = f_pl[b]
        in_maps.append(m)
    return in_maps


def kernel(**inputs):
    from concourse import bass_utils

    if "nc" not in _CACHE:
        _CACHE["nc"] = build_nc()
    nc = _CACHE["nc"]
    in_maps = _host_prep(inputs)
    res = bass_utils.run_bass_kernel_spmd(nc, in_maps, core_ids=list(range(B)))
    out = np.stack([res.results[b]["y"] for b in range(B)], axis=0)
    return out.reshape(B, OUT, H, W).astype(np.float32)


if __name__ == "__main__":
    pass



# revision 6
# speedup vs baseline: 1.0947x; 1.0947x over previous
"""CrossModalAttention TRN2 kernel.

Strategy (data-parallel over batch, one batch element per NeuronCore):
  dir a: q from rgb, k/v from pl;  dir b: q from pl, k/v from rgb.
  Per direction:
    Q  = scale*(Wq @ f_q + bq)        [128 e, N] bf16 (scale folded into W,b)
    K  = Wk @ f_k + bk                [128 e, N] bf16
    VT = (Wv @ f_k)^T                 [N k, 128 e] bf16/fp8 (v-bias folded
                                      into the BN shift host-side)
    per q-tile (512 wide):
      S^T_j = K_j^T @ Q_tile          [128 k, 512 q]  per k-chunk j (PSUM f32)
      E_j   = exp(S^T_j + c)          ScalarE eviction -> bf16/fp8 SBUF
      OT   += VT_j^T @ E_j            [128 e, 512 q]  PSUM accumulate over j
                                      (fp8 path: DoubleRow, 2 chunks/matmul)
      denominators: DVE pair+quad sums, GPSIMD oct sums -> one PE matmul
      per 8 k-chunks: dn += ones^T @ oct
      OT_norm = OT * bcast(1/dn)      reciprocal_approx_fast on DVE; bcast
                                      via Kc=1 rank-1 matmul (e^c cancels)
  y = Wp_a @ OT_a + Wp_b @ OT_b ; out = relu(inv*y + shift)  (BN folded)
"""

import sys

sys.path.insert(0, "/opt/trn_rl_repo")

import numpy as np

B = 8
C = 256
E = 128
OUT = 256
H = W = 64
N = H * W
QW = 512
SCALE = float(E) ** -0.5

OT_FP8 = False      # fp8e4 E/VT + DoubleRow O-matmul
EXP_BIAS = 1.5 if OT_FP8 else 0.0

_CACHE = {}


def _patch_tail_drain(tile_mod, mybir):
    # This walrus build encodes Drain as CTRL_NO_STRUCT with a single
    # sync-wait slot; split the TileContext tail drain's waits across
    # one drain instruction per semaphore.
    if getattr(tile_mod.TileContext, "_drain_patched", False):
        return
    from concourse.vector_clock import ScopedClock

    def _drain_and_barrier(self, tick_clock, wait_clock):
        nc = self.nc
        drain_inst = nc.sync.drain()
        wait_clock.add_sem_waits(
            drain_inst.ins, ScopedClock({None: tick_clock.global_clock})
        )
        si = drain_inst.ins.sync_info
        if si is not None and si.on_wait and len(si.on_wait) > 1:
            waits = list(si.on_wait)
            drain_inst.ins.sync_info = mybir.SyncInfo(
                on_wait=[waits[0]], on_update=list(si.on_update or [])
            )
            for w in waits[1:]:
                d2 = nc.sync.drain()
                d2.ins.sync_info = mybir.SyncInfo(on_wait=[w], on_update=[])
        nc.all_engine_barrier()
        popped = nc._tile_sem_poison_stack.pop()
        assert popped is self._sem_poison
        nc.clear_and_free_semaphores(list(self.sems.allocated().values()))
        nc.all_engine_barrier()

    tile_mod.TileContext._drain_and_barrier = _drain_and_barrier
    tile_mod.TileContext._drain_patched = True


def build_nc(n=N, debug=False):
    """Build the single-core Bass program. n = spatial size (4096 full)."""
    import concourse.bacc as bacc
    import concourse.tile as tile
    from concourse import mybir

    f32 = mybir.dt.float32
    f32r = mybir.dt.float32r
    bf16 = mybir.dt.bfloat16
    e_dt = mybir.dt.float8e4 if OT_FP8 else bf16
    AFT = mybir.ActivationFunctionType

    gj = 2                  # k-chunks per PSUM S-tile / exp instruction
    nqt = n // QW
    nkc = n // 128
    ngrp = nkc // gj        # exp groups per segment
    DN_G = 4                # exp groups per dn matmul (8 k-chunks)
    ndn = ngrp // DN_G

    nc = bacc.Bacc(trn_type="TRN2", target_bir_lowering=False, debug=False)

    def din(name, shape, dt_=f32):
        return nc.dram_tensor(name, shape, dt_, kind="ExternalInput").ap()

    f_a_d = din("f_a", [C, n])        # rgb features (q-side of dir a)
    f_b_d = din("f_b", [C, n])        # pl features
    wq_a_d = din("wq_a", [C, E])      # scale * W_q_rgb^T
    wk_a_d = din("wk_a", [C, E])      # W_k_pl^T
    wv_a_d = din("wv_a", [C, E])      # W_v_pl^T
    wq_b_d = din("wq_b", [C, E])      # scale * W_q_pl^T
    wk_b_d = din("wk_b", [C, E])      # W_k_rgb^T
    wv_b_d = din("wv_b", [C, E])      # W_v_rgb^T
    wp_d = din("wp", [2 * E, OUT])    # w_proj^T
    bq_a_d = din("bq_a", [E, 1])      # scale * b_q_rgb
    bk_a_d = din("bk_a", [E, 1])      # b_k_pl
    bq_b_d = din("bq_b", [E, 1])      # scale * b_q_pl
    bk_b_d = din("bk_b", [E, 1])      # b_k_rgb
    inv_d = din("bn_inv", [OUT, 1])
    shf_d = din("bn_shf", [OUT, 1])
    ones_c_d = din("ones_c", [E, 1], mybir.dt.uint16)   # bf16 bits
    ones_r_d = din("ones_r", [1, E])
    ident_d = din("ident", [E, E], mybir.dt.uint16)     # bf16 bits
    y_d = nc.dram_tensor("y", [OUT, n], f32, kind="ExternalOutput").ap()
    dbg = {}
    if debug:
        for nm in ("dq0", "dk0", "dv0", "dot0", "ddn0"):
            shp = [1, QW] if nm == "ddn0" else ([128, QW] if nm == "dot0" else [128, n])
            dbg[nm] = nc.dram_tensor(nm, shp, f32, kind="ExternalOutput").ap()

    with tile.TileContext(nc) as tc:
        with tc.tile_pool(name="const", bufs=1) as const, \
             tc.tile_pool(name="qkv", bufs=1) as qkv, \
             tc.tile_pool(name="pst", bufs=3, space="PSUM") as pst, \
             tc.tile_pool(name="pot", bufs=1, space="PSUM") as pot, \
             tc.tile_pool(name="pshared", bufs=1, space="PSUM") as pshared:
            # ---- constants (DMA order: critical-path first) ----
            def wload(d, nm):
                t = const.tile([128, 2, E], f32r, name=nm, tag=nm)
                nc.sync.dma_start(t[:], d.rearrange("(c p) e -> p c e", p=128).bitcast(f32r))
                return t

            def vload(d, shape, nm, dt_=None):
                t = const.tile(shape, dt_ or f32, name=nm, tag=nm)
                nc.sync.dma_start(t[:], d.bitcast(dt_) if dt_ else d)
                return t

            wq = {0: wload(wq_a_d, "wqa"), 1: wload(wq_b_d, "wqb")}
            wk = {0: wload(wk_a_d, "wka"), 1: wload(wk_b_d, "wkb")}
            bq = {0: vload(bq_a_d, [E, 1], "bqa"), 1: vload(bq_b_d, [E, 1], "bqb")}
            bk = {0: vload(bk_a_d, [E, 1], "bka"), 1: vload(bk_b_d, [E, 1], "bkb")}

            # ---- per-direction activations ----
            q_sb = {d: qkv.tile([128, n], bf16, tag=f"q{d}", name=f"q_sb{d}") for d in (0, 1)}
            k_sb = {d: qkv.tile([128, n], bf16, tag=f"k{d}", name=f"k_sb{d}") for d in (0, 1)}
            vt_sb = {d: qkv.tile([128, n], e_dt, tag=f"v{d}", name=f"vt_sb{d}") for d in (0, 1)}

            # ---- feature load + projections (feature pool freed after) ----
            with tc.tile_pool(name="feat", bufs=1) as feat:
                fsb = {
                    name: feat.tile([128, 2, n], f32r, tag=f"f{name}",
                                    name=f"f_{name}")
                    for name in ("a", "b")
                }
                npc = max(1, n // 1024)   # ~1MB DMA pieces
                wv = {}
                for pc in range(npc):
                    lo, hi = pc * (n // npc), (pc + 1) * (n // npc)
                    for cc in range(2):
                        for name, d in (("a", f_a_d), ("b", f_b_d)):
                            eng = nc.sync if (cc == 0) else nc.gpsimd
                            eng.dma_start(
                                fsb[name][:, cc, lo:hi],
                                d[cc * 128:(cc + 1) * 128, lo:hi].bitcast(f32r),
                            )
                    if pc == min(1, npc - 1):
                        wv = {0: wload(wv_a_d, "wva"), 1: wload(wv_b_d, "wvb")}
                # late consts (used mid/late in the kernel)
                wp = const.tile([128, 2, OUT], f32r, name="wp", tag="wp")
                nc.sync.dma_start(wp[:], wp_d.rearrange("(c p) e -> p c e", p=128).bitcast(f32r))
                binv = const.tile([128, 2, 1], f32, name="binv", tag="binv")
                nc.sync.dma_start(binv[:], inv_d.rearrange("(c p) e -> p c e", p=128))
                bshf = const.tile([128, 2, 1], f32, name="bshf", tag="bshf")
                nc.sync.dma_start(bshf[:], shf_d.rearrange("(c p) e -> p c e", p=128))
                ones_c = vload(ones_c_d, [E, 1], "onc", bf16)
                ones_r = vload(ones_r_d, [1, E], "onr", f32r)
                ident = vload(ident_d, [E, E], "idt", bf16)

                vtmps = {}
                for d in (0, 1):
                    fq = fsb["a"] if d == 0 else fsb["b"]
                    fk = fsb["b"] if d == 0 else fsb["a"]
                    # Q and K projections -> bf16 [128 e, n], evicted on DVE
                    # (bias add; q-scale folded into weights host-side)
                    for which, wt, bias, dst, src_f in (
                        ("q", wq[d], bq[d], q_sb[d], fq),
                        ("k", wk[d], bk[d], k_sb[d], fk),
                    ):
                        for nt in range(nqt):
                            ps = pst.tile([128, QW], f32, tag="st", name="psqk")
                            for cc in range(2):
                                nc.tensor.matmul(
                                    ps[:],
                                    wt[:, cc, :],
                                    src_f[:, cc, nt * QW:(nt + 1) * QW],
                                    start=(cc == 0),
                                    stop=(cc == 1),
                                )
                            with nc.allow_low_precision(reason="bf16 qk"):
                                nc.vector.tensor_scalar_add(
                                    dst[:, nt * QW:(nt + 1) * QW],
                                    ps[:], bias[:],
                                )
                    # V = Wv @ fk -> bf16 [128 e, n]
                    vtmp = feat.tile([128, n], bf16, tag=f"vtmp{d}",
                                     name=f"vtmp{d}")
                    vtmps[d] = vtmp
                    for nt in range(nqt):
                        ps = pst.tile([128, QW], f32, tag="st", name="psv")
                        for cc in range(2):
                            nc.tensor.matmul(
                                ps[:],
                                wv[d][:, cc, :],
                                fk[:, cc, nt * QW:(nt + 1) * QW],
                                start=(cc == 0),
                                stop=(cc == 1),
                            )
                        with nc.allow_low_precision(reason="bf16 V"):
                            nc.vector.tensor_copy(
                                vtmp[:, nt * QW:(nt + 1) * QW], ps[:]
                            )
                # PE transposes after both dirs' projections (DVE copies hide)
                for d in (0, 1):
                    for g in range(nkc // 4):
                        ps = pst.tile([128, QW], bf16, tag="st", name="psvt")
                        for jj in range(4):
                            kc = 4 * g + jj
                            nc.tensor.transpose(
                                ps[:, jj * 128:(jj + 1) * 128],
                                vtmps[d][:, kc * 128:(kc + 1) * 128],
                                ident[:],
                            )
                        with nc.allow_low_precision(reason="low-prec VT"):
                            nc.vector.tensor_copy(
                                vt_sb[d][:, g * 512:(g + 1) * 512], ps[:]
                            )
            if debug:
                nc.sync.dma_start(dbg["dq0"].bitcast(bf16), q_sb[0][:])
                nc.sync.dma_start(dbg["dk0"].bitcast(bf16), k_sb[0][:])
                nc.sync.dma_start(dbg["dv0"].bitcast(e_dt), vt_sb[0][:])

            # ---- attention + output ----
            with tc.tile_pool(name="sex", bufs=4) as sex, \
                 tc.tile_pool(name="sexs", bufs=6) as sexs, \
                 tc.tile_pool(name="soct", bufs=3) as soct, \
                 tc.tile_pool(name="sot", bufs=3) as sot, \
                 tc.tile_pool(name="smisc", bufs=4) as smisc:

                def emit_S(d, qt, g):
                    """S^T matmuls for one k-chunk group -> st psum tile."""
                    qs = q_sb[d][:, qt * QW:(qt + 1) * QW]
                    st = pst.tile([128, gj, QW], f32, tag="st", name="st")
                    for jj in range(gj):
                        j = gj * g + jj
                        nc.tensor.matmul(
                            st[:, jj, :],
                            k_sb[d][:, j * 128:(j + 1) * 128],
                            qs,
                            start=True, stop=True,
                        )
                    return st

                segs = [(qt, d) for qt in range(nqt) for d in (0, 1)]

                def emit_body(d, qt, st0):
                    """exp + O/dn accumulation for one (qt, d); st0 is the
                    pre-emitted group-0 S tile. Returns (ot, dn) psums."""
                    ot = pot.tile([128, QW], f32, tag="ot", name="ot")
                    dn = pshared.tile([1, QW], f32, tag="sh", name="dn")
                    st_cur = st0
                    quads = []
                    for g in range(ngrp):
                        st_next = emit_S(d, qt, g + 1) if g + 1 < ngrp else None
                        ex = sex.tile([128, gj, QW], e_dt, tag="ex", name="ex")
                        nc.scalar.activation(ex[:], st_cur[:], AFT.Exp,
                                             bias=EXP_BIAS)
                        # denominator tree: pair sums (DVE), quad sums (DVE),
                        # oct sums (GPSIMD) -> one PE matmul per 8 k-chunks
                        exs = sexs.tile([128, QW], bf16, tag="exs", name="exs")
                        with nc.allow_low_precision(reason="bf16 denom"):
                            nc.vector.tensor_add(exs[:], ex[:, 0, :], ex[:, 1, :])
                        quads.append(exs)
                        if OT_FP8:
                            j = gj * g
                            nc.tensor.matmul(
                                ot[:],
                                vt_sb[d][:, j * 128:(j + 2) * 128]
                                .rearrange("p (two f) -> p two f", two=2),
                                ex[:],
                                start=(g == 0), stop=(g == ngrp - 1),
                                perf_mode=mybir.MatmulPerfMode.DoubleRow,
                            )
                        else:
                            for jj in range(gj):
                                j = gj * g + jj
                                nc.tensor.matmul(
                                    ot[:],
                                    vt_sb[d][:, j * 128:(j + 1) * 128],
                                    ex[:, jj, :],
                                    start=(j == 0), stop=(j == nkc - 1),
                                )
                        if g % 2 == 1:
                            q2 = sexs.tile([128, QW], bf16, tag="exs", name="q2")
                            with nc.allow_low_precision(reason="bf16 denom"):
                                nc.vector.tensor_add(q2[:], quads[-2][:], quads[-1][:])
                            quads[-2:] = [q2]
                        if g % DN_G == DN_G - 1:
                            oct_ = soct.tile([128, QW], bf16, tag="oct", name="oct")
                            with nc.allow_low_precision(reason="bf16 denom"):
                                nc.gpsimd.tensor_add(oct_[:], quads[-2][:], quads[-1][:])
                            quads[-2:] = []
                            gd = g // DN_G
                            nc.tensor.matmul(
                                dn[:],
                                ones_c[:],
                                oct_[:],
                                start=(gd == 0), stop=(gd == ndn - 1),
                            )
                        st_cur = st_next
                    if debug and qt == 0 and d == 0:
                        dot_sb = smisc.tile([128, QW], f32, tag="dbgot")
                        nc.vector.tensor_copy(dot_sb[:], ot[:])
                        nc.sync.dma_start(dbg["dot0"][:], dot_sb[:])
                        ddn_sb = smisc.tile([1, QW], f32, tag="dbgdn")
                        nc.vector.tensor_copy(ddn_sb[:], dn[:])
                        nc.sync.dma_start(dbg["ddn0"][:], ddn_sb[:])
                    return ot, dn

                def emit_tail(ot, dn):
                    """softmax normalize -> osb (f32r SBUF)."""
                    rc = smisc.tile([1, QW], f32, tag="rc", name="rc")
                    nc.vector.reciprocal_approx_fast(rc[:], dn[:])
                    rcr = smisc.tile([1, QW], f32r, tag="rcr", name="rcr")
                    with nc.allow_low_precision(reason="f32r recip"):
                        nc.vector.tensor_copy(rcr[:], rc[:])
                    bc_ps = pshared.tile([128, QW], f32, tag="sh", name="bc")
                    nc.tensor.matmul(
                        bc_ps[:], ones_r[:], rcr[:],
                        start=True, stop=True,
                    )
                    bc_sb = smisc.tile([128, QW], f32r, tag="bcs", name="bcs")
                    nc.vector.tensor_copy(bc_sb[:], bc_ps[:])
                    osb = sot.tile([128, QW], f32r, tag="osb", name="osb")
                    nc.vector.tensor_mul(osb[:], ot[:], bc_sb[:])
                    return osb

                def emit_final(qt, ot_sbs):
                    for dch in range(2):
                        yp = pshared.tile([128, QW], f32, tag="sh", name="yp")
                        for d in (0, 1):
                            nc.tensor.matmul(
                                yp[:],
                                wp[:, d, dch * 128:(dch + 1) * 128],
                                ot_sbs[d][:],
                                start=(d == 0), stop=(d == 1),
                            )
                        ysb = smisc.tile([128, QW], f32, tag="ysb", name="ysb")
                        nc.scalar.activation(
                            ysb[:], yp[:], AFT.Relu,
                            bias=bshf[:, dch, :], scale=binv[:, dch, :],
                        )
                        nc.sync.dma_start(
                            y_d[dch * 128:(dch + 1) * 128,
                                qt * QW:(qt + 1) * QW],
                            ysb[:],
                        )

                # software-pipelined over segments: the next segment's first
                # S-group is emitted before the current segment's tail so the
                # PE never idles on the DVE/ACT tail chain.
                st_next0 = emit_S(segs[0][1], segs[0][0], 0)
                pending = {}          # qt -> {d: osb}
                for i, (qt, d) in enumerate(segs):
                    ot, dn = emit_body(d, qt, st_next0)
                    if i + 1 < len(segs):
                        nqt_, nd_ = segs[i + 1]
                        st_next0 = emit_S(nd_, nqt_, 0)
                    osb = emit_tail(ot, dn)
                    pending.setdefault(qt, {})[d] = osb
                    if d == 1:
                        emit_final(qt, pending.pop(qt))
    nc.compile()
    return nc


def _to_bf16_bits(x):
    u = np.ascontiguousarray(x, np.float32).view(np.uint32)
    r = ((u + 0x7FFF + ((u >> 16) & 1)) >> 16).astype(np.uint16)
    return r


def _host_prep(inputs, n=N):
    f_rgb = np.ascontiguousarray(
        inputs["f_rgb"].reshape(B, C, n).astype(np.float32))
    f_pl = np.ascontiguousarray(
        inputs["f_pl"].reshape(B, C, n).astype(np.float32))

    def T(w, scale=1.0):
        return np.ascontiguousarray(scale * np.asarray(w, np.float32).T)

    wp = np.asarray(inputs["w_proj"], np.float32)
    inv = np.asarray(inputs["bn_gamma"], np.float32) / np.sqrt(
        np.asarray(inputs["bn_var"], np.float32) + 1e-5)
    shift = (np.asarray(inputs["bn_beta"], np.float32)
             - np.asarray(inputs["bn_mean"], np.float32) * inv
             + inv * (wp[:, :E] @ np.asarray(inputs["b_v_pl"], np.float32)
                      + wp[:, E:] @ np.asarray(inputs["b_v_rgb"], np.float32)))

    shared = {
        "wq_a": T(inputs["w_q_rgb"], SCALE),
        "wk_a": T(inputs["w_k_pl"]),
        "wv_a": T(inputs["w_v_pl"]),
        "wq_b": T(inputs["w_q_pl"], SCALE),
        "wk_b": T(inputs["w_k_rgb"]),
        "wv_b": T(inputs["w_v_rgb"]),
        "wp": T(wp),
        "bq_a": (SCALE * np.asarray(inputs["b_q_rgb"], np.float32))
        .reshape(E, 1).copy(),
        "bk_a": np.asarray(inputs["b_k_pl"], np.float32).reshape(E, 1).copy(),
        "bq_b": (SCALE * np.asarray(inputs["b_q_pl"], np.float32))
        .reshape(E, 1).copy(),
        "bk_b": np.asarray(inputs["b_k_rgb"], np.float32).reshape(E, 1).copy(),
        "bn_inv": inv.reshape(OUT, 1).copy(),
        "bn_shf": shift.reshape(OUT, 1).copy(),
        "ones_c": _to_bf16_bits(np.ones((E, 1), np.float32)),
        "ones_r": np.ones((1, E), np.float32),
        "ident": _to_bf16_bits(np.eye(E, dtype=np.float32)),
    }
    in_maps = []
    for b in range(B):
        m = dict(shared)
        m["f_a"] = f_rgb[b]
        m["f_b"] = f_pl[b]
        in_maps.append(m)
    return in_maps


def kernel(**inputs):
    from concourse import bass_utils

    if "nc" not in _CACHE:
        _CACHE["nc"] = build_nc()
    nc = _CACHE["nc"]
    in_maps = _host_prep(inputs)
    res = bass_utils.run_bass_kernel_spmd(nc, in_maps, core_ids=list(range(B)))
    out = np.stack([res.results[b]["y"] for b in range(B)], axis=0)
    return out.reshape(B, OUT, H, W).astype(np.float32)


if __name__ == "__main__":
    pass
